# revision 9
# baseline (speedup 1.0000x reference)
"""Trainium2 Bass kernel for nn_Attention_79121887527485.

Multi-head causal attention with ALiBi, B=2 S=2048 D=2048 H=16 DH=128.
Tensor-parallel over heads across 8 NeuronCores: core c owns heads
c (slot 0) and c+8 (slot 1). Each core computes a full [BS, D] partial
of the output projection; the host sums the 8 partials.

Per-core device kernel (all matmuls bf16 with fp32 PSUM accumulation):
  1. QKV: Q^T, K^T in [dh, s] layout, V in [s, dh] layout, from x^T
     tiles streamed from DRAM. Chunk 0 is quarter-interleaved across
     the four (Q/K, lh) accumulation groups in DMA-arrival order;
     warm-up matmuls on constant tiles run during the initial DMA wait
     to release the PE HAM clock throttle.
  2. Attention per (batch, 512-wide q-chunk), two local heads
     interleaved, software-pipelined (LOOK score/exp stages in flight),
     causally skipping k-tiles above the diagonal and (slot 0) k-tiles
     killed by ALiBi decay:
       scores^T[k, q] = (K^T tile).T @ (Q^T chunk)        (PE)
       slot0: += causal/alibi mask or -slope*q row        (DVE)
       slot1: only the 128-wide causal band add (diag)    (DVE)
       P^T = exp(scale*scores^T + bias[p])                (ACT)
         slot0 bias: slope*k - C0 (q-row add carries -slope*q)
         slot1 bias: slope*(k - qc*512) - C1 (per-q-chunk shift; the
         softmax is invariant per (q,head) and slot-1 slopes keep
         slope*(k - qbase) inside fp32 exp range)
       lacc[p,q] += P^T[p,q]    elementwise               (GPSIMD)
       z^T      += (V tile).T @ P^T                       (PE)
     after the last k-tile: lacc -> bf16, one ones-matmul gives the
     denominator l (sum over partitions), z_norm^T = z^T * 1/l.
     The per-tile denominator matmul of the naive scheme is gone: the
     PE does only score+z, and QKV / out-proj matmuls are WOVEN between
     attention iterations so the in-order PE queue never waits for ACT.
  3. Output projection in per-(b, s-chunk) units of 16 o-tiles (written
     as fp16 in ot-pairs, one 256 KB DMA per pair), woven through the
     second half of the program.
"""

import math
from contextlib import ExitStack

import numpy as np
import ml_dtypes

import concourse.bass as bass
import concourse.bacc as bacc
import concourse.tile as tile
from concourse import mybir
from concourse.bass_utils import run_bass_kernel_spmd

B, S, D, H, DH = 2, 2048, 2048, 16, 128
NSC_G = 8                 # global 512-col s-chunks over batch*seq
NCORES = 8
HL = H // NCORES          # 2 local heads per core
BS = B * S                # 4096
HD = HL * DH              # 256 local head dims per core
ND = D // 128             # 16 d-tiles
NQC = S // 512            # 4 q-chunks per batch
SCALE = 1.0 / math.sqrt(DH)
C0 = 14.0                 # slot-0 exp shift (bound for scale*raw_score)
C1 = 20.0                 # slot-1 exp shift (q-chunk-base-relative bias)
NEG = -1.0e6              # raw-units additive causal mask

F32 = mybir.dt.float32
BF16 = mybir.dt.bfloat16
F16 = mybir.dt.float16

_SLOPES = [2.0 ** (-(i + 1) / 2.0) for i in range(H)]

# core c owns heads (c, c + 8). ALiBi decay lets the program skip slot-0
# k-tiles whose whole contribution is < e^-DROP_T relative; the skip set
# must be valid for every core, so it is governed by the smallest slope
# in the slot (head 7 for slot 0; slot 1's head 15 never drops).
DROP_T = 16.0
_SLOT_MIN_SLOPE = [_SLOPES[7], _SLOPES[15]]

LOOK = 2                  # attention score/exp stages in flight


def _heads(c):
    return [c, c + 8]


def _kept_kts(lh, qc):
    kts = []
    for kt in range(4 * qc + 4):
        dist = qc * 512 - (kt * 128 + 127)
        if dist > 0 and _SLOT_MIN_SLOPE[lh] * dist > DROP_T:
            continue
        kts.append(kt)
    return kts


def _build_nc() -> bass.Bass:
    nc = bacc.Bacc("TRN2", target_bir_lowering=False, debug=False, num_devices=NCORES)

    xt_d = nc.dram_tensor("xt", [NSC_G, 128, 8192], BF16, kind="ExternalInput")
    wq_d = [nc.dram_tensor(f"wq{lh}", [128, ND * 128], BF16, kind="ExternalInput")
            for lh in range(HL)]
    wk_d = [nc.dram_tensor(f"wk{lh}", [128, ND * 128], BF16, kind="ExternalInput")
            for lh in range(HL)]
    wv_d = nc.dram_tensor("wv_t", [128, ND * HD], BF16, kind="ExternalInput")
    wo_d = nc.dram_tensor("wo_t", [128, HL * D], BF16, kind="ExternalInput")
    mask_d = nc.dram_tensor("mask", [128, 512], F32, kind="ExternalInput")
    qrow_d = nc.dram_tensor("qrow", [128, NQC * 512], F32, kind="ExternalInput")
    causal_d = nc.dram_tensor("causal", [128, 128], F32, kind="ExternalInput")
    kbias_d = nc.dram_tensor("kbias", [128, 85], F32, kind="ExternalInput")
    out_d = nc.dram_tensor("out_t", [128, ND, NSC_G, 512], F16, kind="ExternalOutput")

    with tile.TileContext(nc) as tc, ExitStack() as ctx:
        const = ctx.enter_context(tc.tile_pool(name="const", bufs=1))
        xq_pool = ctx.enter_context(tc.tile_pool(name="xq", bufs=1))
        xt_pool = ctx.enter_context(tc.tile_pool(name="xt", bufs=2))
        pt_pool = ctx.enter_context(tc.tile_pool(name="pt", bufs=6))
        rc_pool = ctx.enter_context(tc.tile_pool(name="rc", bufs=2))
        oe_pool = ctx.enter_context(tc.tile_pool(name="oe", bufs=8))

        # ---- resident constants / weights ----
        wq_sb = [const.tile([128, ND * 128], BF16, tag=f"wq{lh}", name=f"wq{lh}") for lh in range(HL)]
        wk_sb = [const.tile([128, ND * 128], BF16, tag=f"wk{lh}", name=f"wk{lh}") for lh in range(HL)]
        wv_sb = const.tile([128, ND * HD], BF16, tag="wv")
        wo_sb = const.tile([128, HL * D], BF16, tag="wo")
        mask_sb = const.tile([128, 512], F32, tag="mask")
        qrow_sb = const.tile([128, NQC * 512], F32, tag="qrow")
        causal_sb = const.tile([128, 128], F32, tag="causal")
        kbias_sb = const.tile([128, 85], F32, tag="kbias")
        ones_sb = const.tile([128, 128], BF16, tag="ones")
        warm_sb = const.tile([128, 512], BF16, tag="warm")

        nc.vector.memset(ones_sb[:], 1.0)
        nc.vector.memset(warm_sb[:], 1.0)

        # startup-critical DMAs: Q/K weight halves on sync; x^T quarters
        # of chunk 0 go scalar/gpsimd (emitted in qkv_chunk0)
        for lh in range(HL):
            nc.sync.dma_start(out=wq_sb[lh][:], in_=wq_d[lh].ap())
        for lh in range(HL):
            nc.sync.dma_start(out=wk_sb[lh][:], in_=wk_d[lh].ap())

        # ---- fine-grained resident activations ----
        qt_sb = [[[const.tile([128, 512], BF16, tag=f"qt{lh}{b}{qc}", name=f"qt{lh}{b}{qc}")
                   for qc in range(NQC)] for b in range(B)] for lh in range(HL)]
        kt_sb = [[[const.tile([128, 512], BF16, tag=f"kt{lh}{b}{qc}", name=f"kt{lh}{b}{qc}")
                   for qc in range(NQC)] for b in range(B)] for lh in range(HL)]
        v_sb = [[const.tile([128, HD], BF16, tag=f"v{b}_{st}", name=f"v{b}_{st}")
                 for st in range(16)] for b in range(B)]
        zt_sb = [[[const.tile([128, 512], BF16, tag=f"zt{lh}{b}{qc}", name=f"zt{lh}{b}{qc}")
                   for qc in range(NQC)] for b in range(B)] for lh in range(HL)]

        with ExitStack() as pctx:
            ps_mm = pctx.enter_context(tc.tile_pool(name="ps_mm", bufs=3, space="PSUM"))
            ps_z = pctx.enter_context(tc.tile_pool(name="ps_z", bufs=2, space="PSUM"))
            ps_l = pctx.enter_context(tc.tile_pool(name="ps_l", bufs=2, space="PSUM"))
            ps_warm = pctx.enter_context(tc.tile_pool(name="ps_warm", bufs=1, space="PSUM"))

            # ---- PE warm-up / HAM keep-alive matmuls on a dedicated
            # PSUM bank (no data dependencies, never block real work) ----
            wps = ps_warm.tile([128, 512], F32, tag="warm")

            def ham(n128, n512=0):
                for _ in range(n128):
                    nc.tensor.matmul(wps[:, 0:128], ones_sb[:], warm_sb[:, 0:128],
                                     start=True, stop=True, skip_group_check=True)
                for _ in range(n512):
                    nc.tensor.matmul(wps[:], ones_sb[:], warm_sb[:],
                                     start=True, stop=True, skip_group_check=True)

            ham(40, 8)

            def qkv_chunk0():
                # b=0, scb=0: x^T quarters on two queues; the four (Q/K, lh)
                # accumulation groups consume quarters in arrival order
                xq = [xq_pool.tile([128, 2048], BF16, tag=f"xq{i}", name=f"xq{i}")
                      for i in range(4)]
                nc.scalar.dma_start(out=xq[0][:], in_=xt_d.ap()[0, :, 0:2048])
                nc.scalar.dma_start(out=xq[1][:], in_=xt_d.ap()[0, :, 2048:4096])
                nc.gpsimd.dma_start(out=xq[2][:], in_=xt_d.ap()[0, :, 4096:6144])
                nc.gpsimd.dma_start(out=xq[3][:], in_=xt_d.ap()[0, :, 6144:8192])
                nc.sync.dma_start(out=wv_sb[:], in_=wv_d.ap())
                qtr_order = [0, 2, 1, 3]  # expected DMA arrival order
                dt_order = [q * 4 + r for q in qtr_order for r in range(4)]

                def xsl(dt, lo, size):
                    q, r = divmod(dt, 4)
                    return xq[q][:, r * 512 + lo: r * 512 + lo + size]

                groups = [(wq_sb[0], qt_sb[0][0][0]), (wq_sb[1], qt_sb[1][0][0]),
                          (wk_sb[0], kt_sb[0][0][0]), (wk_sb[1], kt_sb[1][0][0])]
                psums = [ps_mm.tile([128, 512], F32, tag="mm", name=f"qk0ps{gi}")
                         for gi in range(3)]
                psums.append(ps_l.tile([128, 512], F32, tag="l", name="qk0ps3"))
                for qi, qtr in enumerate(qtr_order):
                    if qi:
                        ham(12)  # keep the HAM clock released across DMA waits
                    for gi, (wsb, _) in enumerate(groups):
                        for dt in range(qtr * 4, qtr * 4 + 4):
                            nc.tensor.matmul(
                                psums[gi][:], wsb[:, dt * 128:(dt + 1) * 128],
                                xsl(dt, 0, 512),
                                start=(dt == 0), stop=(dt == ND - 1),
                                skip_group_check=True,
                            )
                for gi, (_, dest) in enumerate(groups):
                    nc.vector.tensor_copy(dest[:], psums[gi][:])

                def v_phase():
                    for ss in range(4):
                        psum = ps_mm.tile([128, HD], F32, tag="mm", name="v0ps")
                        for di, dt in enumerate(dt_order):
                            nc.tensor.matmul(
                                psum[:], xsl(dt, ss * 128, 128), wv_sb[:, dt * HD:(dt + 1) * HD],
                                start=(di == 0), stop=(di == ND - 1),
                            )
                        nc.vector.tensor_copy(v_sb[0][ss][:], psum[:])
                return v_phase

            def qkv_gen(b, scb, skip_v=False):  # uses _xt_halves (defined below)
                # generator: yields roughly every ~430ns of PE work
                sc = b * NQC + scb
                halves = [xt_pool.tile([128, 4096], BF16, tag=f"xt{h}", name=f"xt_{sc}_{h}")
                          for h in range(2)]
                _xt_halves[(b, scb)] = halves
                nc.scalar.dma_start(out=halves[0][:], in_=xt_d.ap()[sc, :, 0:4096])
                nc.gpsimd.dma_start(out=halves[1][:], in_=xt_d.ap()[sc, :, 4096:8192])

                def xsl(dt, lo, size):
                    half = halves[dt // 8]
                    return half[:, (dt % 8) * 512 + lo: (dt % 8) * 512 + lo + size]

                for wsb_pair, dest in ((wq_sb, qt_sb), (wk_sb, kt_sb)):
                    for lh in range(HL):
                        psum = ps_mm.tile([128, 512], F32, tag="mm")
                        for dt in range(ND):
                            nc.tensor.matmul(
                                psum[:], wsb_pair[lh][:, dt * 128:(dt + 1) * 128],
                                xsl(dt, 0, 512),
                                start=(dt == 0), stop=(dt == ND - 1),
                                skip_group_check=True,
                            )
                            if dt % 2 == 1:
                                yield
                        nc.vector.tensor_copy(dest[lh][b][scb][:], psum[:])
                if skip_v:
                    return
                for ss in range(4):
                    psum = ps_mm.tile([128, HD], F32, tag="mm")
                    for dt in range(ND):
                        nc.tensor.matmul(
                            psum[:], xsl(dt, ss * 128, 128), wv_sb[:, dt * HD:(dt + 1) * HD],
                            start=(dt == 0), stop=(dt == ND - 1),
                            skip_group_check=True,
                        )
                        if dt % 4 == 3:
                            yield
                    nc.vector.tensor_copy(v_sb[b][scb * 4 + ss][:], psum[:])

            def v_gen(b, scb):
                # V phase of a chunk whose QK ran with skip_v=True; the
                # xt halves are still resident in the xt pool (bufs=2)
                for ss in range(4):
                    psum = ps_mm.tile([128, HD], F32, tag="mm", name="vps")
                    for dt in range(ND):
                        half = _xt_halves[(b, scb)][dt // 8]
                        nc.tensor.matmul(
                            psum[:], half[:, (dt % 8) * 512 + ss * 128: (dt % 8) * 512 + ss * 128 + 128],
                            wv_sb[:, dt * HD:(dt + 1) * HD],
                            start=(dt == 0), stop=(dt == ND - 1),
                            skip_group_check=True,
                        )
                        if dt % 4 == 3:
                            yield
                    nc.vector.tensor_copy(v_sb[b][scb * 4 + ss][:], psum[:])

            def attn_gen(b, qc):
                lkts = [_kept_kts(0, qc), _kept_kts(1, qc)]
                zps = [ps_z.tile([128, 512], F32, tag="z", name=f"zps{lh}") for lh in range(HL)]
                lps = [ps_l.tile([128, 512], F32, tag="l", name=f"lps{lh}") for lh in range(HL)]
                # l-runs: pairs of full k-tiles (summed on GPSIMD first, one
                # denominator matmul per pair), diagonal tiles individually
                runs = []
                run_of = [{}, {}]
                for lh in range(HL):
                    fulls = [i for i, kt in enumerate(lkts[lh]) if kt - 4 * qc < 0]
                    diags = [i for i, kt in enumerate(lkts[lh]) if kt - 4 * qc >= 0]
                    r = [fulls[k:k + 2] for k in range(0, len(fulls), 2)] + [[i] for i in diags]
                    runs.append(r)
                    for ri, rr in enumerate(r):
                        for pos, i in enumerate(rr):
                            run_of[lh][i] = (ri, pos)
                racc = [{}, {}]
                pend = [{}, {}]   # run_idx -> pt of the run's first member
                pts = {}

                def emit_score(lh, i):
                    kt = lkts[lh][i]
                    j = kt - 4 * qc
                    lo = 128 * j if j >= 0 else 0
                    n = 512 - lo
                    ps = ps_mm.tile([128, 512], F32, tag="mm")
                    nc.tensor.matmul(
                        ps[:, 0:n],
                        kt_sb[lh][b][kt // 4][:, (kt % 4) * 128:(kt % 4) * 128 + 128],
                        qt_sb[lh][b][qc][:, lo:512],
                        start=True, stop=True, skip_group_check=True,
                    )
                    if lh == 0:
                        if j >= 0:
                            nc.vector.tensor_add(ps[:, 0:n], ps[:, 0:n], mask_sb[:, 0:n])
                            bias = kbias_sb[:, 16:17]
                        else:
                            nc.vector.tensor_add(ps[:, 0:n], ps[:, 0:n],
                                                 qrow_sb[:, qc * 512:(qc + 1) * 512])
                            bias = kbias_sb[:, kt:kt + 1]
                    else:
                        if j >= 0:
                            # causal NEG only matters in the first 128 cols
                            nc.vector.tensor_add(ps[:, 0:128], ps[:, 0:128], causal_sb[:])
                            bias = kbias_sb[:, 81 + j:82 + j]
                        else:
                            c = 17 + qc * 16 + kt
                            bias = kbias_sb[:, c:c + 1]
                    pt = pt_pool.tile([128, 512], BF16, tag="pt")
                    nc.scalar.activation(
                        pt[:, 0:n], ps[:, 0:n],
                        mybir.ActivationFunctionType.Exp,
                        bias=bias, scale=SCALE,
                    )
                    ri, pos = run_of[lh][i]
                    rr = runs[lh][ri]
                    if len(rr) == 2:
                        if pos == 0:
                            pend[lh][ri] = pt
                        else:
                            # pair of full tiles: sum their P^T on GPSIMD so
                            # the denominator needs one matmul per pair
                            ra = rc_pool.tile([128, 512], BF16, tag=f"ra{lh}",
                                              name=f"ra{lh}")
                            nc.gpsimd.tensor_add(ra[:], pend[lh].pop(ri)[:], pt[:])
                            racc[lh][ri] = ra
                    pts[(lh, i)] = (pt, lo, n, kt)

                def emit_z(lh, i):
                    pt, lo, n, kt = pts.pop((lh, i))
                    nc.tensor.matmul(
                        zps[lh][:, lo:512],
                        v_sb[b][kt][:, lh * 128:(lh + 1) * 128],
                        pt[:, 0:n],
                        start=(i == 0), stop=(i == len(lkts[lh]) - 1),
                        skip_group_check=True,
                    )
                    ri, pos = run_of[lh][i]
                    rr = runs[lh][ri]
                    if i == rr[-1]:
                        first = (ri == 0)
                        last = (ri == len(runs[lh]) - 1)
                        if len(rr) == 2:
                            nc.tensor.matmul(
                                lps[lh][:], ones_sb[:], racc[lh].pop(ri)[:],
                                start=first, stop=last, skip_group_check=True,
                            )
                        else:
                            nc.tensor.matmul(
                                lps[lh][:, lo:512], ones_sb[:], pt[:, 0:n],
                                start=first, stop=last, skip_group_check=True,
                            )

                seq = []
                i0 = i1 = 0
                while i0 < len(lkts[0]) or i1 < len(lkts[1]):
                    if i1 < len(lkts[1]):
                        seq.append((1, i1)); i1 += 1
                    if i0 < len(lkts[0]):
                        seq.append((0, i0)); i0 += 1
                for t in range(min(LOOK, len(seq))):
                    emit_score(*seq[t])
                for t, (lh, i) in enumerate(seq):
                    emit_z(lh, i)
                    if t + LOOK < len(seq):
                        emit_score(*seq[t + LOOK])
                    yield
                for lh in range(HL):
                    recip = rc_pool.tile([128, 512], F32, tag="rc")
                    scratch = rc_pool.tile([128, 512], F32, tag="rcs")
                    nc.vector.reciprocal_approx_accurate(recip[:], lps[lh][:], scratch[:])
                    nc.vector.tensor_mul(zt_sb[lh][b][qc][:], zps[lh][:], recip[:])
                    yield

            _xt_halves = {}
            n_out = [0]

            def oproj_gen(b, scb):
                # 16 o-tiles for one (batch, s-chunk); fp16 in ot-pairs,
                # one 256 KB DMA per pair on the sync/scalar queues
                sc = b * NQC + scb
                for ot in range(0, 16, 2):
                    o_sb = oe_pool.tile([128, 1024], F16, tag="oe")
                    for half in range(2):
                        psum = ps_mm.tile([128, 512], F32, tag="mm")
                        for lh in range(HL):
                            nc.tensor.matmul(
                                psum[:],
                                wo_sb[:, lh * D + (ot + half) * 128: lh * D + (ot + half) * 128 + 128],
                                zt_sb[lh][b][scb][:],
                                start=(lh == 0), stop=(lh == HL - 1),
                                skip_group_check=True,
                            )
                        if half == 0:
                            nc.scalar.copy(o_sb[:, 0:512], psum[:])
                        else:
                            nc.vector.tensor_copy(o_sb[:, 512:1024], psum[:])
                        yield
                    dma_eng = (nc.sync, nc.scalar)[n_out[0] % 2]
                    dma_eng.dma_start(
                        out=out_d.ap()[:, ot:ot + 2, sc, :],
                        in_=o_sb[:],
                    )
                    n_out[0] += 1

            def drain(gen):
                for _ in gen:
                    pass

            def weave(agen, partners):
                # alternate attention quanta with partner quanta; drain
                # whichever side outlives the other
                def partner_steps():
                    for g in partners:
                        for _ in g:
                            yield
                pgen = partner_steps()
                a_alive = p_alive = True
                while a_alive or p_alive:
                    if a_alive:
                        try:
                            next(agen)
                        except StopIteration:
                            a_alive = False
                    if p_alive:
                        try:
                            next(pgen)
                        except StopIteration:
                            p_alive = False

            # ---- emission ----
            v_phase0 = qkv_chunk0()
            nc.sync.dma_start(out=kbias_sb[:], in_=kbias_d.ap())
            nc.sync.dma_start(out=causal_sb[:], in_=causal_d.ap())
            nc.sync.dma_start(out=mask_sb[:], in_=mask_d.ap())
            ham(12)
            drain(qkv_gen(0, 1, skip_v=True))
            ham(12)
            v_phase0()
            drain(v_gen(0, 1))
            # gated by scalar-engine program order: issue only once the
            # engine reaches this point (keeps them off the startup ramp)
            nc.scalar.dma_start(out=qrow_sb[:], in_=qrow_d.ap())
            weave(attn_gen(0, 0), [qkv_gen(0, 2)])
            nc.scalar.dma_start(out=wo_sb[:], in_=wo_d.ap())
            weave(attn_gen(0, 1), [qkv_gen(0, 3)])
            weave(attn_gen(0, 2), [qkv_gen(1, 0)])
            weave(attn_gen(0, 3), [qkv_gen(1, 1)])
            weave(attn_gen(1, 0), [qkv_gen(1, 2), oproj_gen(0, 0)])
            weave(attn_gen(1, 1), [qkv_gen(1, 3), oproj_gen(0, 1)])
            weave(attn_gen(1, 2), [oproj_gen(0, 2), oproj_gen(0, 3), oproj_gen(1, 0)])
            weave(attn_gen(1, 3), [oproj_gen(1, 1), oproj_gen(1, 2)])
            drain(oproj_gen(1, 3))

    nc.finalize()
    return nc


_NC = None


def _get_nc() -> bass.Bass:
    global _NC
    if _NC is None:
        _NC = _build_nc()
    return _NC


def _make_in_maps(resid_pre, Wq, Wk, Wv, Wo):
    bf = ml_dtypes.bfloat16
    x = np.asarray(resid_pre, np.float32).reshape(BS, D)
    # pre-tiled DMA-friendly layout: xt[sc, p, dt*512 + s] = x[sc*512+s, dt*128+p]
    xt = np.ascontiguousarray(
        x.reshape(NSC_G, 512, D // 128, 128).transpose(0, 3, 2, 1).reshape(NSC_G, 128, 8192)
    ).astype(bf)

    p = np.arange(128)[:, None]
    f = np.arange(512)[None, :]

    Wq = np.asarray(Wq, np.float32)
    Wk = np.asarray(Wk, np.float32)
    Wv = np.asarray(Wv, np.float32)
    Wo = np.asarray(Wo, np.float32)

    causal = np.where(p > f[:, :128], NEG, 0.0).astype(np.float32)

    in_maps = []
    for c in range(NCORES):
        rows = np.r_[c * DH:(c + 1) * DH, (c + 8) * DH:(c + 9) * DH]
        s0 = _SLOPES[c]          # slot 0 slope (head c)
        s1 = _SLOPES[c + 8]      # slot 1 slope (head c+8)

        # slot-0 tables (raw units for the DVE adds; exp bias in kbias)
        qrow = np.zeros((128, NQC * 512), np.float32)
        for qc in range(NQC):
            q = qc * 512 + np.arange(512, dtype=np.float64)
            qrow[:, qc * 512:(qc + 1) * 512] = (-s0 * q / SCALE)[None, :].astype(np.float32)
        mask = ((-s0 * f / SCALE) + np.where(p > f, NEG, 0.0)).astype(np.float32)

        kbias = np.zeros((128, 85), np.float32)
        pp = np.arange(128, dtype=np.float64)
        for kt in range(16):
            kbias[:, kt] = (s0 * (kt * 128 + pp) - C0).astype(np.float32)
        kbias[:, 16] = (s0 * pp - C0).astype(np.float32)
        for qc in range(NQC):
            for kt in range(4 * qc + 4):
                kbias[:, 17 + qc * 16 + kt] = (
                    s1 * (kt * 128 + pp - qc * 512) - C1
                ).astype(np.float32)
        for j in range(4):
            kbias[:, 81 + j] = (s1 * (j * 128 + pp) - C1).astype(np.float32)

        def wsplit(W):
            # [p, lh, dt*128+m] halves of W[rows,:].T
            a = W[rows, :].T.reshape(ND, 128, HL, 128).transpose(1, 2, 0, 3)
            return [np.ascontiguousarray(a[:, lh].reshape(128, ND * 128)).astype(bf)
                    for lh in range(HL)]

        wq0, wq1 = wsplit(Wq)
        wk0, wk1 = wsplit(Wk)
        in_maps.append({
            "xt": xt,
            "wq0": wq0, "wq1": wq1, "wk0": wk0, "wk1": wk1,
            # [p, dt*HD + m] = Wv.T[dt*128+p, m]  (dt-major, both lh)
            "wv_t": np.ascontiguousarray(
                Wv[rows, :].T.reshape(ND, 128, HD).transpose(1, 0, 2).reshape(128, -1)
            ).astype(bf),
            # [p, lh*D + o] = Wo[:, rows].T[lh*128+p, o]
            "wo_t": np.ascontiguousarray(
                Wo[:, rows].T.reshape(HL, 128, D).transpose(1, 0, 2).reshape(128, -1)
            ).astype(bf),
            "mask": mask,
            "qrow": qrow,
            "causal": causal,
            "kbias": kbias,
        })
    return in_maps


def _combine(results) -> np.ndarray:
    acc = np.zeros((128, ND, NSC_G, 512), np.float32)
    for m in results:
        acc += m["out_t"].astype(np.float32)
    # [p, ot, sc, s] -> out^T[ot*128+p, sc*512+s] -> [b, s, o]
    out_t = acc.transpose(1, 0, 2, 3).reshape(D, BS)
    return np.ascontiguousarray(out_t.reshape(D, B, S).transpose(1, 2, 0))


def kernel(resid_pre, Wq, Wk, Wv, Wo):
    nc = _get_nc()
    in_maps = _make_in_maps(resid_pre, Wq, Wk, Wv, Wo)
    res = run_bass_kernel_spmd(nc, in_maps, core_ids=list(range(NCORES)))
    return _combine(res.results)


# revision 10
# speedup vs baseline: 1.0702x; 1.0702x over previous
"""Trainium2 Bass kernel for nn_Attention_79121887527485.

Multi-head causal attention with ALiBi, B=2 S=2048 D=2048 H=16 DH=128.
Tensor-parallel over heads across 8 NeuronCores: core c owns heads
c (slot 0) and c+8 (slot 1). Each core computes a full [BS, D] partial
of the output projection; the host sums the 8 partials.

Per-core device kernel (all matmuls bf16 with fp32 PSUM accumulation):
  1. QKV: Q^T, K^T in [dh, s] layout, V in [s, dh] layout, from x^T
     tiles streamed from DRAM. Chunk 0 is quarter-interleaved across
     the four (Q/K, lh) accumulation groups in DMA-arrival order;
     warm-up matmuls on constant tiles run during the initial DMA wait
     to release the PE HAM clock throttle.
  2. Attention per (batch, 512-wide q-chunk), two local heads
     interleaved, software-pipelined (LOOK score/exp stages in flight),
     causally skipping k-tiles above the diagonal and (slot 0) k-tiles
     killed by ALiBi decay:
       scores^T[k, q] = (K^T tile).T @ (Q^T chunk)        (PE)
       slot0: += causal/alibi mask or -slope*q row        (DVE)
       slot1: only the 128-wide causal band add (diag)    (DVE)
       P^T = exp(scale*scores^T + bias[p])                (ACT)
         slot0 bias: slope*k - C0 (q-row add carries -slope*q)
         slot1 bias: slope*(k - qc*512) - C1 (per-q-chunk shift; the
         softmax is invariant per (q,head) and slot-1 slopes keep
         slope*(k - qbase) inside fp32 exp range)
       lacc[p,q] += P^T[p,q]    elementwise               (GPSIMD)
       z^T      += (V tile).T @ P^T                       (PE)
     after the last k-tile: lacc -> bf16, one ones-matmul gives the
     denominator l (sum over partitions), z_norm^T = z^T * 1/l.
     The per-tile denominator matmul of the naive scheme is gone: the
     PE does only score+z, and QKV / out-proj matmuls are WOVEN between
     attention iterations so the in-order PE queue never waits for ACT.
  3. Output projection in per-(b, s-chunk) units of 16 o-tiles (written
     as fp16 in ot-pairs, one 256 KB DMA per pair), woven through the
     second half of the program.
"""

import math
from contextlib import ExitStack

import numpy as np
import ml_dtypes

import concourse.bass as bass
import concourse.bacc as bacc
import concourse.tile as tile
from concourse import mybir
from concourse.bass_utils import run_bass_kernel_spmd

B, S, D, H, DH = 2, 2048, 2048, 16, 128
NSC_G = 8                 # global 512-col s-chunks over batch*seq
NCORES = 8
HL = H // NCORES          # 2 local heads per core
BS = B * S                # 4096
HD = HL * DH              # 256 local head dims per core
ND = D // 128             # 16 d-tiles
NQC = S // 512            # 4 q-chunks per batch
SCALE = 1.0 / math.sqrt(DH)
C0 = 14.0                 # slot-0 exp shift (bound for scale*raw_score)
C1 = 20.0                 # slot-1 exp shift (q-chunk-base-relative bias)
NEG = -1.0e6              # raw-units additive causal mask

F32 = mybir.dt.float32
BF16 = mybir.dt.bfloat16
F16 = mybir.dt.float16

_SLOPES = [2.0 ** (-(i + 1) / 2.0) for i in range(H)]

# core c owns heads (c, c + 8). ALiBi decay lets the program skip slot-0
# k-tiles whose whole contribution is < e^-DROP_T relative; the skip set
# must be valid for every core, so it is governed by the smallest slope
# in the slot (head 7 for slot 0; slot 1's head 15 never drops).
DROP_T = 12.0
_SLOT_MIN_SLOPE = [_SLOPES[7], _SLOPES[15]]

LOOK = 3                  # attention score/exp stages in flight


def _heads(c):
    return [c, c + 8]


def _kept_kts(lh, qc):
    kts = []
    for kt in range(4 * qc + 4):
        dist = qc * 512 - (kt * 128 + 127)
        if dist > 0 and _SLOT_MIN_SLOPE[lh] * dist > DROP_T:
            continue
        kts.append(kt)
    return kts


def _build_nc() -> bass.Bass:
    nc = bacc.Bacc("TRN2", target_bir_lowering=False, debug=False, num_devices=NCORES)

    xt_d = nc.dram_tensor("xt", [NSC_G, 128, 8192], BF16, kind="ExternalInput")
    wq_d = [nc.dram_tensor(f"wq{lh}", [128, ND * 128], BF16, kind="ExternalInput")
            for lh in range(HL)]
    wk_d = [nc.dram_tensor(f"wk{lh}", [128, ND * 128], BF16, kind="ExternalInput")
            for lh in range(HL)]
    wv_d = nc.dram_tensor("wv_t", [128, ND * HD], BF16, kind="ExternalInput")
    wo_d = nc.dram_tensor("wo_t", [128, HL * D], BF16, kind="ExternalInput")
    mask_d = nc.dram_tensor("mask", [128, 512], F32, kind="ExternalInput")
    qrow_d = nc.dram_tensor("qrow", [128, NQC * 512], F32, kind="ExternalInput")
    causal_d = nc.dram_tensor("causal", [128, 128], F32, kind="ExternalInput")
    kbias_d = nc.dram_tensor("kbias", [128, 85], F32, kind="ExternalInput")
    out_d = nc.dram_tensor("out_t", [128, ND, NSC_G, 512], F16, kind="ExternalOutput")

    with tile.TileContext(nc) as tc, ExitStack() as ctx:
        const = ctx.enter_context(tc.tile_pool(name="const", bufs=1))
        xq_pool = ctx.enter_context(tc.tile_pool(name="xq", bufs=1))
        xt_pool = ctx.enter_context(tc.tile_pool(name="xt", bufs=2))
        pt_pool = ctx.enter_context(tc.tile_pool(name="pt", bufs=6))
        rc_pool = ctx.enter_context(tc.tile_pool(name="rc", bufs=2))
        oe_pool = ctx.enter_context(tc.tile_pool(name="oe", bufs=8))

        # ---- resident constants / weights ----
        wq_sb = [const.tile([128, ND * 128], BF16, tag=f"wq{lh}", name=f"wq{lh}") for lh in range(HL)]
        wk_sb = [const.tile([128, ND * 128], BF16, tag=f"wk{lh}", name=f"wk{lh}") for lh in range(HL)]
        wv_sb = const.tile([128, ND * HD], BF16, tag="wv")
        wo_sb = const.tile([128, HL * D], BF16, tag="wo")
        mask_sb = const.tile([128, 512], F32, tag="mask")
        qrow_sb = const.tile([128, NQC * 512], F32, tag="qrow")
        causal_sb = const.tile([128, 128], F32, tag="causal")
        kbias_sb = const.tile([128, 85], F32, tag="kbias")
        ones_sb = const.tile([128, 128], BF16, tag="ones")
        warm_sb = const.tile([128, 512], BF16, tag="warm")

        nc.vector.memset(ones_sb[:], 1.0)
        nc.vector.memset(warm_sb[:], 1.0)

        # startup-critical DMAs: Q/K weight halves on sync; x^T quarters
        # of chunk 0 go scalar/gpsimd (emitted in qkv_chunk0)
        for lh in range(HL):
            nc.sync.dma_start(out=wq_sb[lh][:], in_=wq_d[lh].ap())
        for lh in range(HL):
            nc.sync.dma_start(out=wk_sb[lh][:], in_=wk_d[lh].ap())

        # ---- fine-grained resident activations ----
        qt_sb = [[[const.tile([128, 512], BF16, tag=f"qt{lh}{b}{qc}", name=f"qt{lh}{b}{qc}")
                   for qc in range(NQC)] for b in range(B)] for lh in range(HL)]
        kt_sb = [[[const.tile([128, 512], BF16, tag=f"kt{lh}{b}{qc}", name=f"kt{lh}{b}{qc}")
                   for qc in range(NQC)] for b in range(B)] for lh in range(HL)]
        v_sb = [[const.tile([128, HD], BF16, tag=f"v{b}_{st}", name=f"v{b}_{st}")
                 for st in range(16)] for b in range(B)]
        zt_sb = [[[const.tile([128, 512], BF16, tag=f"zt{lh}{b}{qc}", name=f"zt{lh}{b}{qc}")
                   for qc in range(NQC)] for b in range(B)] for lh in range(HL)]

        with ExitStack() as pctx:
            ps_mm = pctx.enter_context(tc.tile_pool(name="ps_mm", bufs=4, space="PSUM"))
            ps_z = pctx.enter_context(tc.tile_pool(name="ps_z", bufs=2, space="PSUM"))
            ps_l = pctx.enter_context(tc.tile_pool(name="ps_l", bufs=2, space="PSUM"))

            # ---- PE warm-up / HAM keep-alive matmuls into a ps_z bank
            # (idle until the first attention chunk; no data deps) ----
            wps = ps_z.tile([128, 512], F32, tag="z", name="wps")

            def ham(n128, n512=0):
                for _ in range(n128):
                    nc.tensor.matmul(wps[:, 0:128], ones_sb[:], warm_sb[:, 0:128],
                                     start=True, stop=True, skip_group_check=True)
                for _ in range(n512):
                    nc.tensor.matmul(wps[:], ones_sb[:], warm_sb[:],
                                     start=True, stop=True, skip_group_check=True)

            ham(40, 8)

            def qkv_chunk0():
                # b=0, scb=0: x^T quarters on two queues; the four (Q/K, lh)
                # accumulation groups consume quarters in arrival order
                xq = [xq_pool.tile([128, 2048], BF16, tag=f"xq{i}", name=f"xq{i}")
                      for i in range(4)]
                nc.scalar.dma_start(out=xq[0][:], in_=xt_d.ap()[0, :, 0:2048])
                nc.scalar.dma_start(out=xq[1][:], in_=xt_d.ap()[0, :, 2048:4096])
                nc.gpsimd.dma_start(out=xq[2][:], in_=xt_d.ap()[0, :, 4096:6144])
                nc.gpsimd.dma_start(out=xq[3][:], in_=xt_d.ap()[0, :, 6144:8192])
                qtr_order = [0, 2, 1, 3]  # expected DMA arrival order
                dt_order = [q * 4 + r for q in qtr_order for r in range(4)]

                def xsl(dt, lo, size):
                    q, r = divmod(dt, 4)
                    return xq[q][:, r * 512 + lo: r * 512 + lo + size]

                groups = [(wq_sb[0], qt_sb[0][0][0]), (wq_sb[1], qt_sb[1][0][0]),
                          (wk_sb[0], kt_sb[0][0][0]), (wk_sb[1], kt_sb[1][0][0])]
                psums = [ps_mm.tile([128, 512], F32, tag="mm", name=f"qk0ps{gi}")
                         for gi in range(4)]
                for qi, qtr in enumerate(qtr_order):
                    if qi:
                        ham(12)  # keep the HAM clock released across DMA waits
                    for gi, (wsb, _) in enumerate(groups):
                        for dt in range(qtr * 4, qtr * 4 + 4):
                            nc.tensor.matmul(
                                psums[gi][:], wsb[:, dt * 128:(dt + 1) * 128],
                                xsl(dt, 0, 512),
                                start=(dt == 0), stop=(dt == ND - 1),
                                skip_group_check=True,
                            )
                for gi, (_, dest) in enumerate(groups):
                    nc.vector.tensor_copy(dest[:], psums[gi][:])

                def v_phase():
                    for ss in range(4):
                        psum = ps_mm.tile([128, HD], F32, tag="mm", name="v0ps")
                        for di, dt in enumerate(dt_order):
                            nc.tensor.matmul(
                                psum[:], xsl(dt, ss * 128, 128), wv_sb[:, dt * HD:(dt + 1) * HD],
                                start=(di == 0), stop=(di == ND - 1),
                            )
                        nc.vector.tensor_copy(v_sb[0][ss][:], psum[:])
                return v_phase

            def qkv_gen(b, scb, skip_v=False):  # uses _xt_halves (defined below)
                # generator: yields roughly every ~430ns of PE work
                sc = b * NQC + scb
                halves = [xt_pool.tile([128, 4096], BF16, tag=f"xt{h}", name=f"xt_{sc}_{h}")
                          for h in range(2)]
                _xt_halves[(b, scb)] = halves
                nc.scalar.dma_start(out=halves[0][:], in_=xt_d.ap()[sc, :, 0:4096])
                nc.gpsimd.dma_start(out=halves[1][:], in_=xt_d.ap()[sc, :, 4096:8192])

                def xsl(dt, lo, size):
                    half = halves[dt // 8]
                    return half[:, (dt % 8) * 512 + lo: (dt % 8) * 512 + lo + size]

                for wsb_pair, dest in ((wq_sb, qt_sb), (wk_sb, kt_sb)):
                    for lh in range(HL):
                        psum = ps_mm.tile([128, 512], F32, tag="mm")
                        for dt in range(ND):
                            nc.tensor.matmul(
                                psum[:], wsb_pair[lh][:, dt * 128:(dt + 1) * 128],
                                xsl(dt, 0, 512),
                                start=(dt == 0), stop=(dt == ND - 1),
                                skip_group_check=True,
                            )
                            if dt % 2 == 1:
                                yield
                        nc.vector.tensor_copy(dest[lh][b][scb][:], psum[:])
                if skip_v:
                    return
                for ss in range(4):
                    psum = ps_mm.tile([128, HD], F32, tag="mm")
                    for dt in range(ND):
                        nc.tensor.matmul(
                            psum[:], xsl(dt, ss * 128, 128), wv_sb[:, dt * HD:(dt + 1) * HD],
                            start=(dt == 0), stop=(dt == ND - 1),
                            skip_group_check=True,
                        )
                        if dt % 4 == 3:
                            yield
                    nc.vector.tensor_copy(v_sb[b][scb * 4 + ss][:], psum[:])

            def v_gen(b, scb):
                # V phase of a chunk whose QK ran with skip_v=True; the
                # xt halves are still resident in the xt pool (bufs=2)
                for ss in range(4):
                    psum = ps_mm.tile([128, HD], F32, tag="mm", name="vps")
                    for dt in range(ND):
                        half = _xt_halves[(b, scb)][dt // 8]
                        nc.tensor.matmul(
                            psum[:], half[:, (dt % 8) * 512 + ss * 128: (dt % 8) * 512 + ss * 128 + 128],
                            wv_sb[:, dt * HD:(dt + 1) * HD],
                            start=(dt == 0), stop=(dt == ND - 1),
                            skip_group_check=True,
                        )
                        if dt % 4 == 3:
                            yield
                    nc.vector.tensor_copy(v_sb[b][scb * 4 + ss][:], psum[:])

            def attn_gen(b, qc):
                lkts = [_kept_kts(0, qc), _kept_kts(1, qc)]
                zps = [ps_z.tile([128, 512], F32, tag="z", name=f"zps{lh}") for lh in range(HL)]
                lps = [ps_l.tile([128, 512], F32, tag="l", name=f"lps{lh}") for lh in range(HL)]
                pts = {}

                def emit_score(lh, i):
                    kt = lkts[lh][i]
                    j = kt - 4 * qc
                    lo = 128 * j if j >= 0 else 0
                    n = 512 - lo
                    ps = ps_mm.tile([128, 512], F32, tag="mm")
                    nc.tensor.matmul(
                        ps[:, 0:n],
                        kt_sb[lh][b][kt // 4][:, (kt % 4) * 128:(kt % 4) * 128 + 128],
                        qt_sb[lh][b][qc][:, lo:512],
                        start=True, stop=True, skip_group_check=True,
                    )
                    if lh == 0:
                        if j >= 0:
                            nc.vector.tensor_add(ps[:, 0:n], ps[:, 0:n], mask_sb[:, 0:n])
                            bias = kbias_sb[:, 16:17]
                        else:
                            nc.vector.tensor_add(ps[:, 0:n], ps[:, 0:n],
                                                 qrow_sb[:, qc * 512:(qc + 1) * 512])
                            bias = kbias_sb[:, kt:kt + 1]
                    else:
                        if j >= 0:
                            # causal NEG only matters in the first 128 cols
                            nc.vector.tensor_add(ps[:, 0:128], ps[:, 0:128], causal_sb[:])
                            bias = kbias_sb[:, 81 + j:82 + j]
                        else:
                            c = 17 + qc * 16 + kt
                            bias = kbias_sb[:, c:c + 1]
                    pt = pt_pool.tile([128, 512], BF16, tag="pt")
                    nc.scalar.activation(
                        pt[:, 0:n], ps[:, 0:n],
                        mybir.ActivationFunctionType.Exp,
                        bias=bias, scale=SCALE,
                    )
                    pts[(lh, i)] = (pt, lo, n, kt)

                def emit_z(lh, i):
                    pt, lo, n, kt = pts.pop((lh, i))
                    first = (i == 0)
                    last = (i == len(lkts[lh]) - 1)
                    nc.tensor.matmul(
                        lps[lh][:, lo:512], ones_sb[:], pt[:, 0:n],
                        start=first, stop=last, skip_group_check=True,
                    )
                    nc.tensor.matmul(
                        zps[lh][:, lo:512],
                        v_sb[b][kt][:, lh * 128:(lh + 1) * 128],
                        pt[:, 0:n],
                        start=first, stop=last, skip_group_check=True,
                    )

                seq = []
                i0 = i1 = 0
                while i0 < len(lkts[0]) or i1 < len(lkts[1]):
                    if i1 < len(lkts[1]):
                        seq.append((1, i1)); i1 += 1
                    if i0 < len(lkts[0]):
                        seq.append((0, i0)); i0 += 1
                for t in range(min(LOOK, len(seq))):
                    emit_score(*seq[t])
                for t, (lh, i) in enumerate(seq):
                    emit_z(lh, i)
                    if t + LOOK < len(seq):
                        emit_score(*seq[t + LOOK])
                    yield
                for lh in range(HL):
                    recip = rc_pool.tile([128, 512], F32, tag="rc")
                    scratch = rc_pool.tile([128, 512], F32, tag="rcs")
                    nc.vector.reciprocal_approx_accurate(recip[:], lps[lh][:], scratch[:])
                    nc.vector.tensor_mul(zt_sb[lh][b][qc][:], zps[lh][:], recip[:])
                    yield

            _xt_halves = {}
            n_out = [0]

            def oproj_gen(b, scb):
                # 16 o-tiles for one (batch, s-chunk); fp16 in ot-pairs,
                # one 256 KB DMA per pair on the sync/scalar queues
                sc = b * NQC + scb
                for ot in range(0, 16, 2):
                    o_sb = oe_pool.tile([128, 1024], F16, tag="oe")
                    for half in range(2):
                        psum = ps_mm.tile([128, 512], F32, tag="mm")
                        for lh in range(HL):
                            nc.tensor.matmul(
                                psum[:],
                                wo_sb[:, lh * D + (ot + half) * 128: lh * D + (ot + half) * 128 + 128],
                                zt_sb[lh][b][scb][:],
                                start=(lh == 0), stop=(lh == HL - 1),
                                skip_group_check=True,
                            )
                        if half == 0:
                            nc.scalar.copy(o_sb[:, 0:512], psum[:])
                        else:
                            nc.vector.tensor_copy(o_sb[:, 512:1024], psum[:])
                        yield
                    dma_eng = (nc.sync, nc.scalar)[n_out[0] % 2]
                    dma_eng.dma_start(
                        out=out_d.ap()[:, ot:ot + 2, sc, :],
                        in_=o_sb[:],
                    )
                    n_out[0] += 1

            def drain(gen):
                for _ in gen:
                    pass

            def weave(agen, partners):
                # alternate attention quanta with partner quanta; drain
                # whichever side outlives the other
                def partner_steps():
                    for g in partners:
                        for _ in g:
                            yield
                pgen = partner_steps()
                a_alive = p_alive = True
                while a_alive or p_alive:
                    if a_alive:
                        try:
                            next(agen)
                        except StopIteration:
                            a_alive = False
                    if p_alive:
                        try:
                            next(pgen)
                        except StopIteration:
                            p_alive = False

            # ---- emission ----
            v_phase0 = qkv_chunk0()
            nc.sync.dma_start(out=kbias_sb[:], in_=kbias_d.ap())
            nc.sync.dma_start(out=causal_sb[:], in_=causal_d.ap())
            nc.sync.dma_start(out=mask_sb[:], in_=mask_d.ap())
            ham(12)
            drain(qkv_gen(0, 1, skip_v=True))
            nc.scalar.dma_start(out=wv_sb[:], in_=wv_d.ap())
            ham(12)
            v_phase0()
            drain(v_gen(0, 1))
            # gated by scalar-engine program order: issue only once the
            # engine reaches this point (keeps them off the startup ramp)
            nc.scalar.dma_start(out=qrow_sb[:], in_=qrow_d.ap())
            weave(attn_gen(0, 0), [qkv_gen(0, 2)])
            nc.scalar.dma_start(out=wo_sb[:], in_=wo_d.ap())
            weave(attn_gen(0, 1), [qkv_gen(0, 3)])
            weave(attn_gen(0, 2), [qkv_gen(1, 0)])
            weave(attn_gen(0, 3), [qkv_gen(1, 1)])
            weave(attn_gen(1, 0), [qkv_gen(1, 2), oproj_gen(0, 0)])
            weave(attn_gen(1, 1), [qkv_gen(1, 3), oproj_gen(0, 1)])
            weave(attn_gen(1, 2), [oproj_gen(0, 2), oproj_gen(0, 3), oproj_gen(1, 0)])
            weave(attn_gen(1, 3), [oproj_gen(1, 1), oproj_gen(1, 2)])
            drain(oproj_gen(1, 3))

    nc.finalize()
    return nc


_NC = None


def _get_nc() -> bass.Bass:
    global _NC
    if _NC is None:
        _NC = _build_nc()
    return _NC


def _make_in_maps(resid_pre, Wq, Wk, Wv, Wo):
    bf = ml_dtypes.bfloat16
    x = np.asarray(resid_pre, np.float32).reshape(BS, D)
    # pre-tiled DMA-friendly layout: xt[sc, p, dt*512 + s] = x[sc*512+s, dt*128+p]
    xt = np.ascontiguousarray(
        x.reshape(NSC_G, 512, D // 128, 128).transpose(0, 3, 2, 1).reshape(NSC_G, 128, 8192)
    ).astype(bf)

    p = np.arange(128)[:, None]
    f = np.arange(512)[None, :]

    Wq = np.asarray(Wq, np.float32)
    Wk = np.asarray(Wk, np.float32)
    Wv = np.asarray(Wv, np.float32)
    Wo = np.asarray(Wo, np.float32)

    causal = np.where(p > f[:, :128], NEG, 0.0).astype(np.float32)

    in_maps = []
    for c in range(NCORES):
        rows = np.r_[c * DH:(c + 1) * DH, (c + 8) * DH:(c + 9) * DH]
        s0 = _SLOPES[c]          # slot 0 slope (head c)
        s1 = _SLOPES[c + 8]      # slot 1 slope (head c+8)

        # slot-0 tables (raw units for the DVE adds; exp bias in kbias)
        qrow = np.zeros((128, NQC * 512), np.float32)
        for qc in range(NQC):
            q = qc * 512 + np.arange(512, dtype=np.float64)
            qrow[:, qc * 512:(qc + 1) * 512] = (-s0 * q / SCALE)[None, :].astype(np.float32)
        mask = ((-s0 * f / SCALE) + np.where(p > f, NEG, 0.0)).astype(np.float32)

        kbias = np.zeros((128, 85), np.float32)
        pp = np.arange(128, dtype=np.float64)
        for kt in range(16):
            kbias[:, kt] = (s0 * (kt * 128 + pp) - C0).astype(np.float32)
        kbias[:, 16] = (s0 * pp - C0).astype(np.float32)
        for qc in range(NQC):
            for kt in range(4 * qc + 4):
                kbias[:, 17 + qc * 16 + kt] = (
                    s1 * (kt * 128 + pp - qc * 512) - C1
                ).astype(np.float32)
        for j in range(4):
            kbias[:, 81 + j] = (s1 * (j * 128 + pp) - C1).astype(np.float32)

        def wsplit(W):
            # [p, lh, dt*128+m] halves of W[rows,:].T
            a = W[rows, :].T.reshape(ND, 128, HL, 128).transpose(1, 2, 0, 3)
            return [np.ascontiguousarray(a[:, lh].reshape(128, ND * 128)).astype(bf)
                    for lh in range(HL)]

        wq0, wq1 = wsplit(Wq)
        wk0, wk1 = wsplit(Wk)
        in_maps.append({
            "xt": xt,
            "wq0": wq0, "wq1": wq1, "wk0": wk0, "wk1": wk1,
            # [p, dt*HD + m] = Wv.T[dt*128+p, m]  (dt-major, both lh)
            "wv_t": np.ascontiguousarray(
                Wv[rows, :].T.reshape(ND, 128, HD).transpose(1, 0, 2).reshape(128, -1)
            ).astype(bf),
            # [p, lh*D + o] = Wo[:, rows].T[lh*128+p, o]
            "wo_t": np.ascontiguousarray(
                Wo[:, rows].T.reshape(HL, 128, D).transpose(1, 0, 2).reshape(128, -1)
            ).astype(bf),
            "mask": mask,
            "qrow": qrow,
            "causal": causal,
            "kbias": kbias,
        })
    return in_maps


def _combine(results) -> np.ndarray:
    acc = np.zeros((128, ND, NSC_G, 512), np.float32)
    for m in results:
        acc += m["out_t"].astype(np.float32)
    # [p, ot, sc, s] -> out^T[ot*128+p, sc*512+s] -> [b, s, o]
    out_t = acc.transpose(1, 0, 2, 3).reshape(D, BS)
    return np.ascontiguousarray(out_t.reshape(D, B, S).transpose(1, 2, 0))


def kernel(resid_pre, Wq, Wk, Wv, Wo):
    nc = _get_nc()
    in_maps = _make_in_maps(resid_pre, Wq, Wk, Wv, Wo)
    res = run_bass_kernel_spmd(nc, in_maps, core_ids=list(range(NCORES)))
    return _combine(res.results)


# revision 11
# speedup vs baseline: 1.0963x; 1.0243x over previous
"""Trainium2 Bass kernel for nn_Attention_79121887527485.

Multi-head causal attention with ALiBi, B=2 S=2048 D=2048 H=16 DH=128.
Tensor-parallel over heads across 8 NeuronCores: core c owns heads
c (slot 0) and c+8 (slot 1). Each core computes a full [BS, D] partial
of the output projection; the host sums the 8 partials.

Per-core device kernel (all matmuls bf16 with fp32 PSUM accumulation):
  1. QKV: Q^T, K^T in [dh, s] layout, V in [s, dh] layout, from x^T
     tiles streamed from DRAM. Chunk 0 is quarter-interleaved across
     the four (Q/K, lh) accumulation groups in DMA-arrival order;
     warm-up matmuls on constant tiles run during the initial DMA wait
     to release the PE HAM clock throttle.
  2. Attention per (batch, 512-wide q-chunk), two local heads
     interleaved, software-pipelined (LOOK score/exp stages in flight),
     causally skipping k-tiles above the diagonal and (slot 0) k-tiles
     killed by ALiBi decay:
       scores^T[k, q] = (K^T tile).T @ (Q^T chunk)        (PE)
       slot0: += causal/alibi mask or -slope*q row        (DVE)
       slot1: only the 128-wide causal band add (diag)    (DVE)
       P^T = exp(scale*scores^T + bias[p])                (ACT)
         slot0 bias: slope*k - C0 (q-row add carries -slope*q)
         slot1 bias: slope*(k - qc*512) - C1 (per-q-chunk shift; the
         softmax is invariant per (q,head) and slot-1 slopes keep
         slope*(k - qbase) inside fp32 exp range)
       lacc[p,q] += P^T[p,q]    elementwise               (GPSIMD)
       z^T      += (V tile).T @ P^T                       (PE)
     after the last k-tile: lacc -> bf16, one ones-matmul gives the
     denominator l (sum over partitions), z_norm^T = z^T * 1/l.
     The per-tile denominator matmul of the naive scheme is gone: the
     PE does only score+z, and QKV / out-proj matmuls are WOVEN between
     attention iterations so the in-order PE queue never waits for ACT.
  3. Output projection in per-(b, s-chunk) units of 16 o-tiles (written
     as fp16 in ot-pairs, one 256 KB DMA per pair), woven through the
     second half of the program.
"""

import math
from contextlib import ExitStack

import numpy as np
import ml_dtypes

import concourse.bass as bass
import concourse.bacc as bacc
import concourse.tile as tile
from concourse import mybir
from concourse.bass_utils import run_bass_kernel_spmd

B, S, D, H, DH = 2, 2048, 2048, 16, 128
NSC_G = 8                 # global 512-col s-chunks over batch*seq
NCORES = 8
HL = H // NCORES          # 2 local heads per core
BS = B * S                # 4096
HD = HL * DH              # 256 local head dims per core
ND = D // 128             # 16 d-tiles
NQC = S // 512            # 4 q-chunks per batch
SCALE = 1.0 / math.sqrt(DH)
C0 = 14.0                 # slot-0 exp shift (bound for scale*raw_score)
C1 = 20.0                 # slot-1 exp shift (q-chunk-base-relative bias)
NEG = -1.0e6              # raw-units additive causal mask

F32 = mybir.dt.float32
BF16 = mybir.dt.bfloat16
F16 = mybir.dt.float16

_SLOPES = [2.0 ** (-(i + 1) / 2.0) for i in range(H)]

# core c owns heads (c, c + 8). ALiBi decay lets the program skip slot-0
# k-tiles whose whole contribution is < e^-DROP_T relative; the skip set
# must be valid for every core, so it is governed by the smallest slope
# in the slot (head 7 for slot 0; slot 1's head 15 never drops).
DROP_T = 12.0
_SLOT_MIN_SLOPE = [_SLOPES[7], _SLOPES[15]]

LOOK = 3                  # attention score/exp stages in flight


def _heads(c):
    return [c, c + 8]


def _kept_kts(lh, qc):
    kts = []
    for kt in range(4 * qc + 4):
        dist = qc * 512 - (kt * 128 + 127)
        if dist > 0 and _SLOT_MIN_SLOPE[lh] * dist > DROP_T:
            continue
        kts.append(kt)
    return kts


def _build_nc() -> bass.Bass:
    nc = bacc.Bacc("TRN2", target_bir_lowering=False, debug=False, num_devices=NCORES)

    xt_d = nc.dram_tensor("xt", [NSC_G, 128, 8192], BF16, kind="ExternalInput")
    wq_d = [nc.dram_tensor(f"wq{lh}", [128, ND * 128], BF16, kind="ExternalInput")
            for lh in range(HL)]
    wk_d = [nc.dram_tensor(f"wk{lh}", [128, ND * 128], BF16, kind="ExternalInput")
            for lh in range(HL)]
    wv_d = nc.dram_tensor("wv_t", [128, ND * HD], BF16, kind="ExternalInput")
    wo_d = nc.dram_tensor("wo_t", [128, HL * D], BF16, kind="ExternalInput")
    mask_d = nc.dram_tensor("mask", [128, 512], F32, kind="ExternalInput")
    qrow_d = nc.dram_tensor("qrow", [128, NQC * 512], F32, kind="ExternalInput")
    causal_d = nc.dram_tensor("causal", [128, 128], F32, kind="ExternalInput")
    kbias_d = nc.dram_tensor("kbias", [128, 85], F32, kind="ExternalInput")
    out_d = nc.dram_tensor("out_t", [128, ND, NSC_G, 512], F16, kind="ExternalOutput")

    with tile.TileContext(nc) as tc, ExitStack() as ctx:
        const = ctx.enter_context(tc.tile_pool(name="const", bufs=1))
        xq_pool = ctx.enter_context(tc.tile_pool(name="xq", bufs=1))
        xt_pool = ctx.enter_context(tc.tile_pool(name="xt", bufs=2))
        pt_pool = ctx.enter_context(tc.tile_pool(name="pt", bufs=6))
        rc_pool = ctx.enter_context(tc.tile_pool(name="rc", bufs=2))
        oe_pool = ctx.enter_context(tc.tile_pool(name="oe", bufs=8))

        # ---- resident constants / weights ----
        wq_sb = [const.tile([128, ND * 128], BF16, tag=f"wq{lh}", name=f"wq{lh}") for lh in range(HL)]
        wk_sb = [const.tile([128, ND * 128], BF16, tag=f"wk{lh}", name=f"wk{lh}") for lh in range(HL)]
        wv_sb = const.tile([128, ND * HD], BF16, tag="wv")
        wo_sb = const.tile([128, HL * D], BF16, tag="wo")
        mask_sb = const.tile([128, 512], F32, tag="mask")
        qrow_sb = const.tile([128, NQC * 512], F32, tag="qrow")
        causal_sb = const.tile([128, 128], F32, tag="causal")
        kbias_sb = const.tile([128, 85], F32, tag="kbias")
        ones_sb = const.tile([128, 128], BF16, tag="ones")
        warm_sb = const.tile([128, 512], BF16, tag="warm")

        nc.vector.memset(ones_sb[:], 1.0)
        nc.vector.memset(warm_sb[:], 1.0)

        # startup-critical DMAs split across all three queues so the
        # weights and x^T quarters arrive in consumption order
        for lh in range(HL):
            nc.sync.dma_start(out=wq_sb[lh][:], in_=wq_d[lh].ap())
        for lh in range(HL):
            nc.gpsimd.dma_start(out=wk_sb[lh][:], in_=wk_d[lh].ap())
        nc.sync.dma_start(out=wv_sb[:], in_=wv_d.ap())

        # ---- fine-grained resident activations ----
        qt_sb = [[[const.tile([128, 512], BF16, tag=f"qt{lh}{b}{qc}", name=f"qt{lh}{b}{qc}")
                   for qc in range(NQC)] for b in range(B)] for lh in range(HL)]
        kt_sb = [[[const.tile([128, 512], BF16, tag=f"kt{lh}{b}{qc}", name=f"kt{lh}{b}{qc}")
                   for qc in range(NQC)] for b in range(B)] for lh in range(HL)]
        v_sb = [[const.tile([128, HD], BF16, tag=f"v{b}_{st}", name=f"v{b}_{st}")
                 for st in range(16)] for b in range(B)]
        zt_sb = [[[const.tile([128, 512], BF16, tag=f"zt{lh}{b}{qc}", name=f"zt{lh}{b}{qc}")
                   for qc in range(NQC)] for b in range(B)] for lh in range(HL)]

        with ExitStack() as pctx:
            ps_mm = pctx.enter_context(tc.tile_pool(name="ps_mm", bufs=4, space="PSUM"))
            ps_z = pctx.enter_context(tc.tile_pool(name="ps_z", bufs=2, space="PSUM"))
            ps_l = pctx.enter_context(tc.tile_pool(name="ps_l", bufs=2, space="PSUM"))

            # ---- PE warm-up / HAM keep-alive matmuls into a ps_z bank
            # (idle until the first attention chunk; no data deps) ----
            wps = ps_z.tile([128, 512], F32, tag="z", name="wps")

            def ham(n128, n512=0):
                for _ in range(n128):
                    nc.tensor.matmul(wps[:, 0:128], ones_sb[:], warm_sb[:, 0:128],
                                     start=True, stop=True, skip_group_check=True)
                for _ in range(n512):
                    nc.tensor.matmul(wps[:], ones_sb[:], warm_sb[:],
                                     start=True, stop=True, skip_group_check=True)

            ham(40, 8)

            def qkv_chunk0():
                # b=0, scb=0: x^T quarters on two queues; the four (Q/K, lh)
                # accumulation groups consume quarters in arrival order
                xq = [xq_pool.tile([128, 2048], BF16, tag=f"xq{i}", name=f"xq{i}")
                      for i in range(4)]
                nc.scalar.dma_start(out=xq[0][:], in_=xt_d.ap()[0, :, 0:2048])
                nc.scalar.dma_start(out=xq[1][:], in_=xt_d.ap()[0, :, 2048:4096])
                nc.gpsimd.dma_start(out=xq[2][:], in_=xt_d.ap()[0, :, 4096:6144])
                nc.gpsimd.dma_start(out=xq[3][:], in_=xt_d.ap()[0, :, 6144:8192])
                qtr_order = [0, 2, 1, 3]  # expected DMA arrival order
                dt_order = [q * 4 + r for q in qtr_order for r in range(4)]

                def xsl(dt, lo, size):
                    q, r = divmod(dt, 4)
                    return xq[q][:, r * 512 + lo: r * 512 + lo + size]

                groups = [(wq_sb[0], qt_sb[0][0][0]), (wq_sb[1], qt_sb[1][0][0]),
                          (wk_sb[0], kt_sb[0][0][0]), (wk_sb[1], kt_sb[1][0][0])]
                psums = [ps_mm.tile([128, 512], F32, tag="mm", name=f"qk0ps{gi}")
                         for gi in range(4)]
                # V psums borrow the (startup-idle) z/l banks so the V
                # matmuls can interleave with the Q/K quarter batches
                vpsums = [ps_z.tile([128, HD], F32, tag="z", name=f"v0ps{ss}")
                          for ss in range(2)]
                vpsums += [ps_l.tile([128, HD], F32, tag="l", name=f"v0ps{ss + 2}")
                           for ss in range(2)]
                for qi, qtr in enumerate(qtr_order):
                    for gi, (wsb, _) in enumerate(groups):
                        for dt in range(qtr * 4, qtr * 4 + 4):
                            nc.tensor.matmul(
                                psums[gi][:], wsb[:, dt * 128:(dt + 1) * 128],
                                xsl(dt, 0, 512),
                                start=(dt == 0), stop=(dt == ND - 1),
                                skip_group_check=True,
                            )
                    for ss in range(4):
                        for dt in range(qtr * 4, qtr * 4 + 4):
                            nc.tensor.matmul(
                                vpsums[ss][:], xsl(dt, ss * 128, 128),
                                wv_sb[:, dt * HD:(dt + 1) * HD],
                                start=(qi == 0 and dt == qtr * 4),
                                stop=(qi == 3 and dt == qtr * 4 + 3),
                                skip_group_check=True,
                            )
                for gi, (_, dest) in enumerate(groups):
                    nc.vector.tensor_copy(dest[:], psums[gi][:])
                for ss in range(4):
                    nc.vector.tensor_copy(v_sb[0][ss][:], vpsums[ss][:])

            def qkv_gen(b, scb, skip_v=False):  # uses _xt_halves (defined below)
                # generator: yields roughly every ~430ns of PE work
                sc = b * NQC + scb
                halves = [xt_pool.tile([128, 4096], BF16, tag=f"xt{h}", name=f"xt_{sc}_{h}")
                          for h in range(2)]
                _xt_halves[(b, scb)] = halves
                nc.scalar.dma_start(out=halves[0][:], in_=xt_d.ap()[sc, :, 0:4096])
                nc.gpsimd.dma_start(out=halves[1][:], in_=xt_d.ap()[sc, :, 4096:8192])

                def xsl(dt, lo, size):
                    half = halves[dt // 8]
                    return half[:, (dt % 8) * 512 + lo: (dt % 8) * 512 + lo + size]

                for wsb_pair, dest in ((wq_sb, qt_sb), (wk_sb, kt_sb)):
                    for lh in range(HL):
                        psum = ps_mm.tile([128, 512], F32, tag="mm")
                        for dt in range(ND):
                            nc.tensor.matmul(
                                psum[:], wsb_pair[lh][:, dt * 128:(dt + 1) * 128],
                                xsl(dt, 0, 512),
                                start=(dt == 0), stop=(dt == ND - 1),
                                skip_group_check=True,
                            )
                            if dt % 2 == 1:
                                yield
                        nc.vector.tensor_copy(dest[lh][b][scb][:], psum[:])
                if skip_v:
                    return
                for ss in range(4):
                    psum = ps_mm.tile([128, HD], F32, tag="mm")
                    for dt in range(ND):
                        nc.tensor.matmul(
                            psum[:], xsl(dt, ss * 128, 128), wv_sb[:, dt * HD:(dt + 1) * HD],
                            start=(dt == 0), stop=(dt == ND - 1),
                            skip_group_check=True,
                        )
                        if dt % 4 == 3:
                            yield
                    nc.vector.tensor_copy(v_sb[b][scb * 4 + ss][:], psum[:])

            def v_gen(b, scb):
                # V phase of a chunk whose QK ran with skip_v=True; the
                # xt halves are still resident in the xt pool (bufs=2)
                for ss in range(4):
                    psum = ps_mm.tile([128, HD], F32, tag="mm", name="vps")
                    for dt in range(ND):
                        half = _xt_halves[(b, scb)][dt // 8]
                        nc.tensor.matmul(
                            psum[:], half[:, (dt % 8) * 512 + ss * 128: (dt % 8) * 512 + ss * 128 + 128],
                            wv_sb[:, dt * HD:(dt + 1) * HD],
                            start=(dt == 0), stop=(dt == ND - 1),
                            skip_group_check=True,
                        )
                        if dt % 4 == 3:
                            yield
                    nc.vector.tensor_copy(v_sb[b][scb * 4 + ss][:], psum[:])

            def attn_gen(b, qc):
                lkts = [_kept_kts(0, qc), _kept_kts(1, qc)]
                zps = [ps_z.tile([128, 512], F32, tag="z", name=f"zps{lh}") for lh in range(HL)]
                lps = [ps_l.tile([128, 512], F32, tag="l", name=f"lps{lh}") for lh in range(HL)]
                pts = {}

                def emit_score(lh, i):
                    kt = lkts[lh][i]
                    j = kt - 4 * qc
                    lo = 128 * j if j >= 0 else 0
                    n = 512 - lo
                    ps = ps_mm.tile([128, 512], F32, tag="mm")
                    nc.tensor.matmul(
                        ps[:, 0:n],
                        kt_sb[lh][b][kt // 4][:, (kt % 4) * 128:(kt % 4) * 128 + 128],
                        qt_sb[lh][b][qc][:, lo:512],
                        start=True, stop=True, skip_group_check=True,
                    )
                    if lh == 0:
                        if j >= 0:
                            nc.vector.tensor_add(ps[:, 0:n], ps[:, 0:n], mask_sb[:, 0:n])
                            bias = kbias_sb[:, 16:17]
                        else:
                            nc.vector.tensor_add(ps[:, 0:n], ps[:, 0:n],
                                                 qrow_sb[:, qc * 512:(qc + 1) * 512])
                            bias = kbias_sb[:, kt:kt + 1]
                    else:
                        if j >= 0:
                            # causal NEG only matters in the first 128 cols
                            nc.vector.tensor_add(ps[:, 0:128], ps[:, 0:128], causal_sb[:])
                            bias = kbias_sb[:, 81 + j:82 + j]
                        else:
                            c = 17 + qc * 16 + kt
                            bias = kbias_sb[:, c:c + 1]
                    pt = pt_pool.tile([128, 512], BF16, tag="pt")
                    nc.scalar.activation(
                        pt[:, 0:n], ps[:, 0:n],
                        mybir.ActivationFunctionType.Exp,
                        bias=bias, scale=SCALE,
                    )
                    pts[(lh, i)] = (pt, lo, n, kt)

                def emit_z(lh, i):
                    pt, lo, n, kt = pts.pop((lh, i))
                    first = (i == 0)
                    last = (i == len(lkts[lh]) - 1)
                    nc.tensor.matmul(
                        lps[lh][:, lo:512], ones_sb[:], pt[:, 0:n],
                        start=first, stop=last, skip_group_check=True,
                    )
                    nc.tensor.matmul(
                        zps[lh][:, lo:512],
                        v_sb[b][kt][:, lh * 128:(lh + 1) * 128],
                        pt[:, 0:n],
                        start=first, stop=last, skip_group_check=True,
                    )

                seq = []
                i0 = i1 = 0
                while i0 < len(lkts[0]) or i1 < len(lkts[1]):
                    if i1 < len(lkts[1]):
                        seq.append((1, i1)); i1 += 1
                    if i0 < len(lkts[0]):
                        seq.append((0, i0)); i0 += 1
                for t in range(min(LOOK, len(seq))):
                    emit_score(*seq[t])
                for t, (lh, i) in enumerate(seq):
                    emit_z(lh, i)
                    if t + LOOK < len(seq):
                        emit_score(*seq[t + LOOK])
                    yield
                for lh in range(HL):
                    recip = rc_pool.tile([128, 512], F32, tag="rc")
                    scratch = rc_pool.tile([128, 512], F32, tag="rcs")
                    nc.vector.reciprocal_approx_accurate(recip[:], lps[lh][:], scratch[:])
                    nc.vector.tensor_mul(zt_sb[lh][b][qc][:], zps[lh][:], recip[:])
                    yield

            _xt_halves = {}
            n_out = [0]

            def oproj_gen(b, scb):
                # 16 o-tiles for one (batch, s-chunk); fp16 in ot-pairs,
                # one 256 KB DMA per pair on the sync/scalar queues
                sc = b * NQC + scb
                for ot in range(0, 16, 2):
                    o_sb = oe_pool.tile([128, 1024], F16, tag="oe")
                    for half in range(2):
                        psum = ps_mm.tile([128, 512], F32, tag="mm")
                        for lh in range(HL):
                            nc.tensor.matmul(
                                psum[:],
                                wo_sb[:, lh * D + (ot + half) * 128: lh * D + (ot + half) * 128 + 128],
                                zt_sb[lh][b][scb][:],
                                start=(lh == 0), stop=(lh == HL - 1),
                                skip_group_check=True,
                            )
                        if half == 0:
                            nc.scalar.copy(o_sb[:, 0:512], psum[:])
                        else:
                            nc.vector.tensor_copy(o_sb[:, 512:1024], psum[:])
                        yield
                    dma_eng = (nc.sync, nc.scalar)[n_out[0] % 2]
                    dma_eng.dma_start(
                        out=out_d.ap()[:, ot:ot + 2, sc, :],
                        in_=o_sb[:],
                    )
                    n_out[0] += 1

            def drain(gen):
                for _ in gen:
                    pass

            def weave(agen, partners):
                # alternate attention quanta with partner quanta; drain
                # whichever side outlives the other
                def partner_steps():
                    for g in partners:
                        for _ in g:
                            yield
                pgen = partner_steps()
                a_alive = p_alive = True
                while a_alive or p_alive:
                    if a_alive:
                        try:
                            next(agen)
                        except StopIteration:
                            a_alive = False
                    if p_alive:
                        try:
                            next(pgen)
                        except StopIteration:
                            p_alive = False

            # ---- emission ----
            qkv_chunk0()
            nc.sync.dma_start(out=kbias_sb[:], in_=kbias_d.ap())
            nc.sync.dma_start(out=causal_sb[:], in_=causal_d.ap())
            nc.sync.dma_start(out=mask_sb[:], in_=mask_d.ap())
            ham(12)
            drain(qkv_gen(0, 1, skip_v=True))
            ham(12)
            drain(v_gen(0, 1))
            # gated by scalar-engine program order: issue only once the
            # engine reaches this point (keeps them off the startup ramp)
            nc.scalar.dma_start(out=qrow_sb[:], in_=qrow_d.ap())
            weave(attn_gen(0, 0), [qkv_gen(0, 2)])
            nc.scalar.dma_start(out=wo_sb[:], in_=wo_d.ap())
            weave(attn_gen(0, 1), [qkv_gen(0, 3)])
            weave(attn_gen(0, 2), [qkv_gen(1, 0)])
            weave(attn_gen(0, 3), [qkv_gen(1, 1)])
            weave(attn_gen(1, 0), [qkv_gen(1, 2), oproj_gen(0, 0)])
            weave(attn_gen(1, 1), [qkv_gen(1, 3), oproj_gen(0, 1)])
            weave(attn_gen(1, 2), [oproj_gen(0, 2), oproj_gen(0, 3), oproj_gen(1, 0)])
            weave(attn_gen(1, 3), [oproj_gen(1, 1), oproj_gen(1, 2)])
            drain(oproj_gen(1, 3))

    nc.finalize()
    return nc


_NC = None


def _get_nc() -> bass.Bass:
    global _NC
    if _NC is None:
        _NC = _build_nc()
    return _NC


def _make_in_maps(resid_pre, Wq, Wk, Wv, Wo):
    bf = ml_dtypes.bfloat16
    x = np.asarray(resid_pre, np.float32).reshape(BS, D)
    # pre-tiled DMA-friendly layout: xt[sc, p, dt*512 + s] = x[sc*512+s, dt*128+p]
    xt = np.ascontiguousarray(
        x.reshape(NSC_G, 512, D // 128, 128).transpose(0, 3, 2, 1).reshape(NSC_G, 128, 8192)
    ).astype(bf)

    p = np.arange(128)[:, None]
    f = np.arange(512)[None, :]

    Wq = np.asarray(Wq, np.float32)
    Wk = np.asarray(Wk, np.float32)
    Wv = np.asarray(Wv, np.float32)
    Wo = np.asarray(Wo, np.float32)

    causal = np.where(p > f[:, :128], NEG, 0.0).astype(np.float32)

    in_maps = []
    for c in range(NCORES):
        rows = np.r_[c * DH:(c + 1) * DH, (c + 8) * DH:(c + 9) * DH]
        s0 = _SLOPES[c]          # slot 0 slope (head c)
        s1 = _SLOPES[c + 8]      # slot 1 slope (head c+8)

        # slot-0 tables (raw units for the DVE adds; exp bias in kbias)
        qrow = np.zeros((128, NQC * 512), np.float32)
        for qc in range(NQC):
            q = qc * 512 + np.arange(512, dtype=np.float64)
            qrow[:, qc * 512:(qc + 1) * 512] = (-s0 * q / SCALE)[None, :].astype(np.float32)
        mask = ((-s0 * f / SCALE) + np.where(p > f, NEG, 0.0)).astype(np.float32)

        kbias = np.zeros((128, 85), np.float32)
        pp = np.arange(128, dtype=np.float64)
        for kt in range(16):
            kbias[:, kt] = (s0 * (kt * 128 + pp) - C0).astype(np.float32)
        kbias[:, 16] = (s0 * pp - C0).astype(np.float32)
        for qc in range(NQC):
            for kt in range(4 * qc + 4):
                kbias[:, 17 + qc * 16 + kt] = (
                    s1 * (kt * 128 + pp - qc * 512) - C1
                ).astype(np.float32)
        for j in range(4):
            kbias[:, 81 + j] = (s1 * (j * 128 + pp) - C1).astype(np.float32)

        def wsplit(W):
            # [p, lh, dt*128+m] halves of W[rows,:].T
            a = W[rows, :].T.reshape(ND, 128, HL, 128).transpose(1, 2, 0, 3)
            return [np.ascontiguousarray(a[:, lh].reshape(128, ND * 128)).astype(bf)
                    for lh in range(HL)]

        wq0, wq1 = wsplit(Wq)
        wk0, wk1 = wsplit(Wk)
        in_maps.append({
            "xt": xt,
            "wq0": wq0, "wq1": wq1, "wk0": wk0, "wk1": wk1,
            # [p, dt*HD + m] = Wv.T[dt*128+p, m]  (dt-major, both lh)
            "wv_t": np.ascontiguousarray(
                Wv[rows, :].T.reshape(ND, 128, HD).transpose(1, 0, 2).reshape(128, -1)
            ).astype(bf),
            # [p, lh*D + o] = Wo[:, rows].T[lh*128+p, o]
            "wo_t": np.ascontiguousarray(
                Wo[:, rows].T.reshape(HL, 128, D).transpose(1, 0, 2).reshape(128, -1)
            ).astype(bf),
            "mask": mask,
            "qrow": qrow,
            "causal": causal,
            "kbias": kbias,
        })
    return in_maps


def _combine(results) -> np.ndarray:
    acc = np.zeros((128, ND, NSC_G, 512), np.float32)
    for m in results:
        acc += m["out_t"].astype(np.float32)
    # [p, ot, sc, s] -> out^T[ot*128+p, sc*512+s] -> [b, s, o]
    out_t = acc.transpose(1, 0, 2, 3).reshape(D, BS)
    return np.ascontiguousarray(out_t.reshape(D, B, S).transpose(1, 2, 0))


def kernel(resid_pre, Wq, Wk, Wv, Wo):
    nc = _get_nc()
    in_maps = _make_in_maps(resid_pre, Wq, Wk, Wv, Wo)
    res = run_bass_kernel_spmd(nc, in_maps, core_ids=list(range(NCORES)))
    return _combine(res.results)
